# revision 1
# baseline (speedup 1.0000x reference)
"""Trainium2 Bass kernel for nn_CategorySpecificInitNet (moe_routing).

kernel(**inputs) takes the FULL unsharded inputs (keys as in
reference.setup_inputs()) and returns the FULL [B, 128] float32 output.

Strategy — expert-parallel, per the spec sharding hint's dispatch-by-category
alternative:
  - the host sharding layer dispatches rows to cores by category (the
    "all-to-all dispatch by category" of expert-parallel, realized where
    all sharding happens in this harness): rows are stably sorted by
    cat_idx and core k receives category k's rows, zero-padded to a
    static per-core capacity (max category count rounded up to the
    512-row tile size);
  - every core runs the shared encoder plus exactly ONE decoder (its
    category's), so no routing, masking, or gather happens per row —
    the decoder FLOPs drop 8x vs computing all decoders densely;
  - the encoder's linear third layer is constant-folded into the
    decoder's first layer on the host (W_f = We3 @ Wd1_k,
    b_f = Wd1_k^T be3 + bd1_k — exact algebra, ~0.1% of the FLOPs),
    removing one full matmul stage from the device;
  - all compute is feature-major [features(partitions), rows(free)], so
    no transposes are ever needed on device (the host passes features
    pre-transposed); outputs come back [128, cap] and the host
    inverse-permutes rows during unsharding.
  - per-core row tiles of 512; the decoder stages are software-pipelined
    one tile behind the encoder so the PE never waits on ACT/DVE
    relu latency.

Matmuls run in float32r (fp32 storage, full PE rate at N=512, ~tf32-grade
multiply precision on HW — measured ~3e-4 max rel error vs the fp32
reference, 17x better than bf16 at the same PE throughput).
"""
import sys

for _p in ("/opt/trn_rl_repo",):
    if _p not in sys.path:
        sys.path.append(_p)

import numpy as np

import concourse.bass as bass
import concourse.bacc as bacc
import concourse.mybir as mybir
import concourse.tile as tile
from concourse import bass_utils

FR = mybir.dt.float32r
F32 = mybir.dt.float32
Alu = mybir.AluOpType
ActF = mybir.ActivationFunctionType

B, C, H1, H2, HO = 32768, 768, 512, 256, 256
DH, LAT, K = 256, 128, 8
N_CORES = 8
TILE = 512
# bias_all columns: be1[4] be2[2] bf[2](=Wd1^T be3 + bd1) bd2[2] bd3[1]
OB1, OB2, OD1, OD2, OD3 = 0, 4, 6, 8, 10
NBIAS = 11


def _build_nc(cap, tile_n=512, ps_w_bufs=6, dp_bufs=2, ps_o_bufs=2, split=(3, 3), tail_pos=1, ap_bufs=3, fp_bufs=2):
    assert cap % 256 == 0
    tiles = [tile_n] * (cap // tile_n)
    if cap % tile_n:
        tiles.insert(tail_pos if tail_pos is not None else len(tiles),
                     cap % tile_n)
    offs = [sum(tiles[:i]) for i in range(len(tiles))]
    nt = len(tiles)
    nc = bacc.Bacc(name="catnet_ep")

    fT = nc.dram_tensor("fT", (C, cap), FR, kind="ExternalInput")
    we1 = nc.dram_tensor("we1", (C, H1), FR, kind="ExternalInput")
    we2 = nc.dram_tensor("we2", (H1, H2), FR, kind="ExternalInput")
    wd1 = nc.dram_tensor("wd1", (H2, DH), FR, kind="ExternalInput")  # We3 @ Wd1
    wd2 = nc.dram_tensor("wd2", (DH, DH), FR, kind="ExternalInput")
    wd3 = nc.dram_tensor("wd3", (DH, LAT), FR, kind="ExternalInput")
    bias_all = nc.dram_tensor("bias_all", (128, NBIAS), F32, kind="ExternalInput")
    out = nc.dram_tensor("out", (LAT, cap), F32, kind="ExternalOutput")

    nC, nH1, nH2, nHO, nDH = C // 128, H1 // 128, H2 // 128, HO // 128, DH // 128

    with tile.TileContext(nc) as tc:
        with (
            tc.tile_pool(name="wp", bufs=1) as wp,
            tc.tile_pool(name="fp", bufs=fp_bufs) as fp,
            tc.tile_pool(name="ap", bufs=ap_bufs) as ap,
            tc.tile_pool(name="dp", bufs=dp_bufs) as dp,
            tc.tile_pool(name="ps_w", bufs=ps_w_bufs, space="PSUM") as ps_w,
            tc.tile_pool(name="ps_o", bufs=ps_o_bufs, space="PSUM") as ps_o,
        ):
            # we1 first, in two half-tensor DMAs: per-HWDGE-DMA queue issue
            # costs ~0.6us, so 2 DMAs beats 6 for total latency while still
            # letting the first L1 matmuls start after the first half lands
            we1_t = wp.tile([128, nC, H1], FR, tag="we1")
            we1_r = we1.rearrange("(c p) h -> p c h", p=128)
            c0 = 0
            for w in split:
                nc.gpsimd.dma_start(we1_t[:, c0:c0 + w, :], we1_r[:, c0:c0 + w, :])
                c0 += w
            bias_t = wp.tile([128, NBIAS], F32, tag="bias")
            nc.gpsimd.dma_start(bias_t[:], bias_all[:])
            we2_t = wp.tile([128, nH1, H2], FR, tag="we2")
            nc.gpsimd.dma_start(we2_t[:], we2.rearrange("(c p) h -> p c h", p=128))
            wd1_t = wp.tile([128, nH2, DH], FR, tag="wd1")
            nc.gpsimd.dma_start(wd1_t[:], wd1.rearrange("(c p) d -> p c d", p=128))
            wd2_t = wp.tile([128, nDH, DH], FR, tag="wd2")
            nc.gpsimd.dma_start(wd2_t[:], wd2.rearrange("(c p) d -> p c d", p=128))
            wd3_t = wp.tile([128, nDH, LAT], FR, tag="wd3")
            nc.gpsimd.dma_start(wd3_t[:], wd3.rearrange("(c p) d -> p c d", p=128))

            def emit_enc(t):
                tn = tiles[t]
                sl = slice(offs[t], offs[t] + tn)
                ftb_fl = fp.tile([128, nC, tile_n], FR, tag="ft")
                ftb = ftb_fl[:, :, :tn]
                if t == 0:
                    fr = fT.rearrange("(c p) b -> p c b", p=128)[:, :, sl]
                    c0 = 0
                    for w in split:
                        nc.sync.dma_start(ftb[:, c0:c0 + w, :], fr[:, c0:c0 + w, :])
                        c0 += w
                else:
                    # alternate queues so consecutive feature tiles stream in
                    # parallel instead of serializing on one HWDGE queue
                    eng = nc.sync if t % 2 == 0 else nc.gpsimd
                    eng.dma_start(
                        ftb[:], fT.rearrange("(c p) b -> p c b", p=128)[:, :, sl])
                pwsl = slice(0, tn)
                a1 = []
                for m in range(nH1):
                    pw_fl = ps_w.tile([128, tile_n], F32, tag="pw")
                    pw = pw_fl[:, :tn]
                    for c in range(nC):
                        nc.tensor.matmul(pw[:], we1_t[:, c, bass.ts(m, 128)],
                                         ftb[:, c, :],
                                         start=(c == 0), stop=(c == nC - 1))
                    x_fl = ap.tile([128, tile_n], FR, tag=f"a1_{m}")
                    x = x_fl[:, :tn]
                    nc.scalar.activation(x[:], pw[:], ActF.Relu,
                                         bias=bias_t[:, OB1 + m:OB1 + m + 1])
                    a1.append(x)
                a2 = []
                for m in range(nH2):
                    pw_fl = ps_w.tile([128, tile_n], F32, tag="pw")
                    pw = pw_fl[:, :tn]
                    for c in range(nH1):
                        nc.tensor.matmul(pw[:], we2_t[:, c, bass.ts(m, 128)], a1[c][:],
                                         start=(c == 0), stop=(c == nH1 - 1))
                    x_fl = ap.tile([128, tile_n], FR, tag=f"a2_{m}")
                    x = x_fl[:, :tn]
                    if m % 2 == 0:
                        nc.vector.tensor_scalar(x[:], pw[:],
                                                bias_t[:, OB2 + m:OB2 + m + 1],
                                                0.0, Alu.add, Alu.max)
                    else:
                        nc.scalar.activation(x[:], pw[:], ActF.Relu,
                                             bias=bias_t[:, OB2 + m:OB2 + m + 1])
                    a2.append(x)
                return a2

            def emit_d1(t, h):
                tn = tiles[t]
                d1 = []
                for m in range(nDH):
                    pw_fl = ps_w.tile([128, tile_n], F32, tag="pw")
                    pw = pw_fl[:, :tn]
                    for c in range(nH2):
                        nc.tensor.matmul(pw[:], wd1_t[:, c, bass.ts(m, 128)], h[c][:],
                                         start=(c == 0), stop=(c == nH2 - 1))
                    x_fl = dp.tile([128, tile_n], FR, tag=f"d1_{m}")
                    x = x_fl[:, :tn]
                    if m % 2 == 1:
                        nc.vector.tensor_scalar(x[:], pw[:],
                                                bias_t[:, OD1 + m:OD1 + m + 1],
                                                0.0, Alu.add, Alu.max)
                    else:
                        nc.scalar.activation(x[:], pw[:], ActF.Relu,
                                             bias=bias_t[:, OD1 + m:OD1 + m + 1])
                    d1.append(x)
                return d1

            def emit_d2_d3_store(t, d1):
                tn = tiles[t]
                d2 = []
                for m in range(nDH):
                    pw_fl = ps_w.tile([128, tile_n], F32, tag="pw")
                    pw = pw_fl[:, :tn]
                    for c in range(nDH):
                        nc.tensor.matmul(pw[:], wd2_t[:, c, bass.ts(m, 128)], d1[c][:],
                                         start=(c == 0), stop=(c == nDH - 1))
                    x_fl = dp.tile([128, tile_n], FR, tag=f"d2_{m}")
                    x = x_fl[:, :tn]
                    bb = bias_t[:, OD2 + m:OD2 + m + 1]
                    if m % 2 == 0:
                        nc.vector.tensor_scalar(x[:], pw[:], bb, 0.0, Alu.add, Alu.max)
                    else:
                        nc.scalar.activation(x[:], pw[:], ActF.Relu, bias=bb)
                    d2.append(x)
                po_fl = ps_o.tile([128, tile_n], F32, tag="out")
                po = po_fl[:, :tn]
                for c in range(nDH):
                    nc.tensor.matmul(po[:], wd3_t[:, c, :], d2[c][:],
                                     start=(c == 0), stop=(c == nDH - 1))
                osb_fl = ap.tile([128, tile_n], F32, tag="osb")
                osb = osb_fl[:, :tn]
                nc.scalar.activation(osb[:], po[:], ActF.Identity,
                                     bias=bias_t[:, OD3:OD3 + 1])
                nc.gpsimd.dma_start(out[:, offs[t]:offs[t] + tn], osb[:])

            # decoder runs one tile behind the encoder: PE order per step is
            # [enc t][d2/d3 t-1][d1 t], hiding ACT/DVE relu latency behind
            # independent matmuls
            pend = None
            for t in range(nt):
                h = emit_enc(t)
                if pend is not None:
                    emit_d2_d3_store(pend[0], pend[1])
                d1 = emit_d1(t, h)
                pend = (t, d1)
            emit_d2_d3_store(pend[0], pend[1])

    nc.finalize()
    return nc


def _pack_inputs(features, We1, be1, We2, be2, We3, be3,
                 Wd1, bd1, Wd2, bd2, Wd3, bd3, cat_idx, cap):
    """Dispatch rows to cores by category (expert-parallel sharding)."""
    features = np.asarray(features, np.float32)
    cat = np.asarray(cat_idx).astype(np.int64)
    order = np.argsort(cat, kind="stable")
    counts = np.bincount(cat, minlength=N_CORES)
    starts = np.zeros(N_CORES + 1, np.int64)
    np.cumsum(counts, out=starts[1:])

    def chunkcols(b):
        b = np.asarray(b, np.float32).reshape(-1)
        return b.reshape(-1, 128).T

    enc = dict(
        we1=np.asarray(We1, np.float32), we2=np.asarray(We2, np.float32),
    )
    We3f = np.asarray(We3, np.float32)
    be3f = np.asarray(be3, np.float32)
    maps, rows_per_core = [], []
    for k in range(N_CORES):
        rows = order[starts[k]:starts[k + 1]]
        rows_per_core.append(rows)
        f = np.zeros((cap, C), np.float32)
        f[:len(rows)] = features[rows]
        bias_all = np.zeros((128, NBIAS), np.float32)
        bias_all[:, OB1:OB1 + 4] = chunkcols(be1)
        bias_all[:, OB2:OB2 + 2] = chunkcols(be2)
        wd1k = np.asarray(Wd1, np.float32)[k]
        bias_all[:, OD1:OD1 + 2] = chunkcols(
            wd1k.T @ be3f + np.asarray(bd1, np.float32)[k])
        bias_all[:, OD2:OD2 + 2] = chunkcols(np.asarray(bd2, np.float32)[k])
        bias_all[:, OD3:OD3 + 1] = chunkcols(np.asarray(bd3, np.float32)[k])
        m = dict(enc)
        m["fT"] = np.ascontiguousarray(f.T)
        m["wd1"] = We3f @ wd1k  # encoder L3 folded into decoder layer 1
        m["wd2"] = np.asarray(Wd2, np.float32)[k]
        m["wd3"] = np.asarray(Wd3, np.float32)[k]
        m["bias_all"] = bias_all
        maps.append(m)
    return maps, rows_per_core


_NC_CACHE = {}


def _get_nc(cap=4352):
    if cap not in _NC_CACHE:
        _NC_CACHE[cap] = _build_nc(cap)
    return _NC_CACHE[cap]


def kernel(**inputs) -> np.ndarray:
    cat = np.asarray(inputs["cat_idx"]).astype(np.int64)
    counts = np.bincount(cat, minlength=N_CORES)
    cap = max(256, int(-(-counts.max() // 256) * 256))
    maps, rows_per_core = _pack_inputs(**inputs, cap=cap)
    nc = _get_nc(cap)
    res = bass_utils.run_bass_kernel_spmd(nc, maps, core_ids=list(range(N_CORES)))
    latent = np.zeros((B, LAT), np.float32)
    for k, r in enumerate(res.results):
        rows = rows_per_core[k]
        latent[rows] = r["out"][:, :len(rows)].T
    return latent



# revision 15
# speedup vs baseline: 1.0696x; 1.0696x over previous
"""Trainium2 Bass kernel for nn_CategorySpecificInitNet (moe_routing).

kernel(**inputs) takes the FULL unsharded inputs (keys as in
reference.setup_inputs()) and returns the FULL [B, 128] float32 output.

Strategy — expert-parallel, per the spec sharding hint's dispatch-by-category
alternative:
  - the host sharding layer dispatches rows to cores by category: rows are
    stably sorted by cat_idx and core k receives category k's rows,
    zero-padded to a static per-core capacity (max category count rounded
    up to 256);
  - every core runs the shared encoder plus exactly ONE decoder (its
    category's), so the decoder FLOPs drop 8x vs computing all decoders
    densely;
  - the encoder's linear third layer is constant-folded into the decoder's
    first layer on the host (W_f = We3 @ Wd1_k, b_f = Wd1_k^T be3 + bd1_k —
    exact algebra, ~0.1% of the FLOPs);
  - all compute is feature-major [features(partitions), rows(free)], so no
    transposes are needed on device (the host passes features
    pre-transposed); outputs come back [128, cap] and the host
    inverse-permutes rows during unsharding.

Schedule (all timings against the TimelineSim cost model):
  - the PE clock ramps to full speed only after ~3us of continuous busy;
    a chain of tiny warmup matmuls on a memset tile anchors the ramp at
    t~0.3us so every real matmul runs at 2.4GHz;
  - layer 1 runs in bf16 (features + We1 only; <2e-3 output error): halves
    the feature DMA and the first-chunk latency;
  - the first feature tile and We1 stream in 128-row chunks on two HWDGE
    queues (SP + ACT) in parallel, so the first real matmul issues at
    ~3.5us instead of ~7.3us; L1 accumulates c-outer/m-inner so compute
    starts as soon as chunk 0 lands;
  - per-step PE order is [L1 t][d2+d3 t-1][L2 t][d1 t]: the previous tile's
    decoder tail hides the L1 activation latency;
  - the 256-row remainder tile runs LAST, and its final activation + store
    are split in half on the idle SP HWDGE queue, shrinking the
    end-of-kernel tail.
"""
import sys

for _p in ("/opt/trn_rl_repo",):
    if _p not in sys.path:
        sys.path.append(_p)

import numpy as np
import ml_dtypes

import concourse.bass as bass
import concourse.bacc as bacc
import concourse.mybir as mybir
import concourse.tile as tile
from concourse import bass_utils

FR = mybir.dt.float32r
F32 = mybir.dt.float32
BF16 = mybir.dt.bfloat16
Alu = mybir.AluOpType
ActF = mybir.ActivationFunctionType

B, C, H1, H2, HO = 32768, 768, 512, 256, 256
DH, LAT, K = 256, 128, 8
N_CORES = 8
TILE = 512
# bias_all columns: be1[4] be2[2] bf[2](=Wd1^T be3 + bd1) bd2[2] bd3[1]
OB1, OB2, OD1, OD2, OD3 = 0, 4, 6, 8, 10
NBIAS = 11


def _build_nc(cap, tile_n=512, n_warm=13, ps_w_bufs=6, dp_bufs=3, ps_o_bufs=2,
              ap_bufs=3, fp_bufs=3):
    assert cap % 256 == 0
    tiles = [tile_n] * (cap // tile_n)
    if cap % tile_n:
        tiles.append(cap % tile_n)  # remainder tile LAST (shortest tail)
    offs = [sum(tiles[:i]) for i in range(len(tiles))]
    nt = len(tiles)
    nc = bacc.Bacc(name="catnet_ep")

    fT = nc.dram_tensor("fT", (C, cap), BF16, kind="ExternalInput")
    we1 = nc.dram_tensor("we1", (C, H1), BF16, kind="ExternalInput")
    we2 = nc.dram_tensor("we2", (H1, H2), BF16, kind="ExternalInput")
    wd1 = nc.dram_tensor("wd1", (H2, DH), FR, kind="ExternalInput")  # We3 @ Wd1
    wd2 = nc.dram_tensor("wd2", (DH, DH), FR, kind="ExternalInput")
    wd3 = nc.dram_tensor("wd3", (DH, LAT), FR, kind="ExternalInput")
    bias_all = nc.dram_tensor("bias_all", (128, NBIAS), F32, kind="ExternalInput")
    out = nc.dram_tensor("out", (LAT, cap), F32, kind="ExternalOutput")

    nC, nH1, nH2, nDH = C // 128, H1 // 128, H2 // 128, DH // 128

    with tile.TileContext(nc) as tc:
        with (
            tc.tile_pool(name="wp", bufs=1) as wp,
            tc.tile_pool(name="fp", bufs=fp_bufs) as fp,
            tc.tile_pool(name="ap", bufs=ap_bufs) as ap,
            tc.tile_pool(name="dp", bufs=dp_bufs) as dp,
            tc.tile_pool(name="ps_w", bufs=ps_w_bufs, space="PSUM") as ps_w,
            tc.tile_pool(name="ps_o", bufs=ps_o_bufs, space="PSUM") as ps_o,
        ):
            # --- PE warmup: anchor the p-state ramp at ~t=0.3us so real
            # matmuls (arriving ~3.5us) run at full clock ---
            warm_sb = wp.tile([128, 256], F32, tag="warm_sb")
            nc.gpsimd.memset(warm_sb[:], 0.0)
            warm_fr = warm_sb.bitcast(FR)
            for _ in range(n_warm):
                warm_ps = ps_o.tile([128, 256], F32, tag="out")
                nc.tensor.matmul(warm_ps[:], warm_fr[:, :128], warm_fr[:],
                                 start=True, stop=True)

            # --- weight + bias DMAs ---
            # queue order controls the (serialized) DMA pipe: ACT carries We1
            # chunks then the small tail weights; SP carries the t0 feature
            # chunks then We2/Wd1 (needed next) then the feature tiles.
            # Tensors are declared here; dma_starts are issued interleaved
            # with emit_l1(0) below so nothing preempts the startup chunks.
            we1_t = wp.tile([128, nC, H1], BF16, tag="we1")
            we1_r = we1.rearrange("(c p) h -> p c h", p=128)
            c0 = 0
            for w in (1, 2, 3):
                nc.scalar.dma_start(we1_t[:, c0:c0 + w, :], we1_r[:, c0:c0 + w, :])
                c0 += w
            bias_t = wp.tile([128, NBIAS], F32, tag="bias")
            nc.scalar.dma_start(bias_t[:], bias_all[:])
            we2_t = wp.tile([128, nH1, H2], BF16, tag="we2")
            wd1_t = wp.tile([128, nH2, DH], FR, tag="wd1")
            wd2_t = wp.tile([128, nDH, DH], FR, tag="wd2")
            wd3_t = wp.tile([128, nDH, LAT], FR, tag="wd3")

            def emit_weight_dmas():
                nc.sync.dma_start(we2_t[:],
                                  we2.rearrange("(c p) h -> p c h", p=128))
                nc.sync.dma_start(wd1_t[:],
                                  wd1.rearrange("(c p) d -> p c d", p=128))

            def emit_weight_dmas2():
                nc.sync.dma_start(wd2_t[:],
                                  wd2.rearrange("(c p) d -> p c d", p=128))
                nc.sync.dma_start(wd3_t[:],
                                  wd3.rearrange("(c p) d -> p c d", p=128))

            nact = [0]  # alternate pointwise ops between ACT and DVE

            def pact(x, pw, bias_col, relu, split=False):
                if split:
                    # halve latency: ACT and DVE process half-columns each
                    h = x.shape[-1] // 2
                    if relu:
                        nc.scalar.activation(x[:, :h], pw[:, :h], ActF.Relu,
                                             bias=bias_col)
                        nc.vector.tensor_scalar(x[:, h:], pw[:, h:], bias_col,
                                                0.0, Alu.add, Alu.max)
                    else:
                        nc.scalar.activation(x[:, :h], pw[:, :h], ActF.Identity,
                                             bias=bias_col)
                        nc.vector.tensor_scalar(x[:, h:], pw[:, h:], bias_col,
                                                0.0, Alu.add, Alu.bypass)
                    return
                n = nact[0]
                nact[0] += 1
                if relu:
                    if n % 2 == 0:
                        nc.scalar.activation(x[:], pw[:], ActF.Relu,
                                             bias=bias_col)
                    else:
                        nc.vector.tensor_scalar(x[:], pw[:], bias_col,
                                                0.0, Alu.add, Alu.max)
                else:
                    nc.scalar.activation(x[:], pw[:], ActF.Identity,
                                         bias=bias_col)

            def emit_l1(t):
                tn = tiles[t]
                sl = slice(offs[t], offs[t] + tn)
                ftb_fl = fp.tile([128, nC, tile_n], BF16, tag="ft")
                ftb = ftb_fl[:, :, :tn]
                fr = fT.rearrange("(c p) b -> p c b", p=128)[:, :, sl]
                if t == 0:
                    # chunked DMAs: L1 starts after chunk 0 (~0.4us xfer);
                    # (1,2,3) split balances per-DMA issue overhead (~0.7us)
                    # against time-to-first-matmul
                    c0 = 0
                    for w in (1, 2, 3):
                        nc.sync.dma_start(ftb[:, c0:c0 + w, :], fr[:, c0:c0 + w, :])
                        c0 += w
                    emit_weight_dmas()  # behind the t0 chunks on the pipe
                elif t == 1:
                    nc.sync.dma_start(ftb[:, :3, :], fr[:, :3, :])
                    nc.sync.dma_start(ftb[:, 3:, :], fr[:, 3:, :])
                    emit_weight_dmas2()  # wd2/wd3 queue behind t1's chunks
                else:
                    nc.sync.dma_start(ftb[:], fr)
                # c-outer / m-inner: 4 concurrent PSUM accumulation groups,
                # compute starts as soon as c-chunk 0 lands
                pws = []
                for m in range(nH1):
                    pw_fl = ps_w.tile([128, tile_n], F32, tag="pw", name=f"pw1_{m}")
                    pws.append(pw_fl[:, :tn])
                for c in range(nC):
                    for m in range(nH1):
                        nc.tensor.matmul(pws[m][:], we1_t[:, c, bass.ts(m, 128)],
                                         ftb[:, c, :],
                                         start=(c == 0), stop=(c == nC - 1))
                a1 = []
                for m in range(nH1):
                    x = ap.tile([128, tile_n], BF16, tag=f"a1_{m}", name="a1x")[:, :tn]
                    pact(x, pws[m], bias_t[:, OB1 + m:OB1 + m + 1], True)
                    a1.append(x)
                return a1

            def emit_l2(t, a1, split=False):
                tn = tiles[t]
                a2 = []
                for m in range(nH2):
                    pw = ps_w.tile([128, tile_n], F32, tag="pw", name="pw")[:, :tn]
                    for c in range(nH1):
                        nc.tensor.matmul(pw[:], we2_t[:, c, bass.ts(m, 128)],
                                         a1[c][:],
                                         start=(c == 0), stop=(c == nH1 - 1))
                    x = ap.tile([128, tile_n], FR, tag=f"a2_{m}", name="a2x")[:, :tn]
                    pact(x, pw, bias_t[:, OB2 + m:OB2 + m + 1], True, split)
                    a2.append(x)
                return a2

            def emit_d1(t, h, split=False):
                tn = tiles[t]
                d1 = []
                for m in range(nDH):
                    pw = ps_w.tile([128, tile_n], F32, tag="pw", name="pw")[:, :tn]
                    for c in range(nH2):
                        nc.tensor.matmul(pw[:], wd1_t[:, c, bass.ts(m, 128)],
                                         h[c][:],
                                         start=(c == 0), stop=(c == nH2 - 1))
                    x = dp.tile([128, tile_n], FR, tag=f"d1_{m}", name="d1x")[:, :tn]
                    pact(x, pw, bias_t[:, OD1 + m:OD1 + m + 1], True, split)
                    d1.append(x)
                return d1

            def emit_d2(t, d1, split=False):
                tn = tiles[t]
                d2 = []
                for m in range(nDH):
                    pw = ps_w.tile([128, tile_n], F32, tag="pw", name="pw")[:, :tn]
                    for c in range(nDH):
                        nc.tensor.matmul(pw[:], wd2_t[:, c, bass.ts(m, 128)],
                                         d1[c][:],
                                         start=(c == 0), stop=(c == nDH - 1))
                    x = dp.tile([128, tile_n], FR, tag=f"d2_{m}", name="d2x")[:, :tn]
                    pact(x, pw, bias_t[:, OD2 + m:OD2 + m + 1], True, split)
                    d2.append(x)
                return d2

            def emit_d3_store(t, d2, split=False):
                tn = tiles[t]
                last = t == nt - 1
                po = ps_o.tile([128, tile_n], F32, tag="out", name="po")[:, :tn]
                for c in range(nDH):
                    nc.tensor.matmul(po[:], wd3_t[:, c, :], d2[c][:],
                                     start=(c == 0), stop=(c == nDH - 1))
                osb = ap.tile([128, tile_n], F32, tag="osb", name="osb")[:, :tn]
                pact(osb, po, bias_t[:, OD3:OD3 + 1], False, split)
                # the last store rides the idle SP HWDGE queue (fast issue)
                eng = nc.sync if last else nc.gpsimd
                eng.dma_start(out[:, offs[t]:offs[t] + tn], osb[:])

            def emit_d2_d3_store(t, d1):
                emit_d3_store(t, emit_d2(t, d1))

            # decoder tail of tile t-2 runs between L1(t) and L2(t): those
            # matmuls are dependency-free and hide the L1 activation latency;
            # depth 2 keeps ready PE work in the endgame so the final tile's
            # act latencies stay covered
            pend = []
            for t in range(nt - 1):
                a1 = emit_l1(t)
                if len(pend) >= 2:
                    emit_d2_d3_store(*pend.pop(0))
                a2 = emit_l2(t, a1)
                pend.append((t, emit_d1(t, a2)))
            # endgame: L1(T); d2d3(T-2); L2(T); d2d3(T-1); d1(T); d2d3(T) —
            # the pending full tiles' decoders cover the tail tile's act
            # latencies, and only the short tail store trails the last matmul
            tl_ = nt - 1
            a1 = emit_l1(tl_)
            emit_d2_d3_store(*pend.pop(0))
            a2 = emit_l2(tl_, a1)
            emit_d2_d3_store(*pend.pop(0))
            d1 = emit_d1(tl_, a2)
            emit_d2_d3_store(tl_, d1)

    nc.finalize()
    return nc


def _pack_inputs(features, We1, be1, We2, be2, We3, be3,
                 Wd1, bd1, Wd2, bd2, Wd3, bd3, cat_idx, cap):
    """Dispatch rows to cores by category (expert-parallel sharding)."""
    features = np.asarray(features, np.float32)
    cat = np.asarray(cat_idx).astype(np.int64)
    order = np.argsort(cat, kind="stable")
    counts = np.bincount(cat, minlength=N_CORES)
    starts = np.zeros(N_CORES + 1, np.int64)
    np.cumsum(counts, out=starts[1:])

    def chunkcols(b):
        b = np.asarray(b, np.float32).reshape(-1)
        return b.reshape(-1, 128).T

    enc = dict(
        we1=np.asarray(We1, np.float32).astype(ml_dtypes.bfloat16),
        we2=np.asarray(We2, np.float32).astype(ml_dtypes.bfloat16),
    )
    We3f = np.asarray(We3, np.float32)
    be3f = np.asarray(be3, np.float32)
    maps, rows_per_core = [], []
    for k in range(N_CORES):
        rows = order[starts[k]:starts[k + 1]]
        rows_per_core.append(rows)
        f = np.zeros((cap, C), np.float32)
        f[:len(rows)] = features[rows]
        bias_all = np.zeros((128, NBIAS), np.float32)
        bias_all[:, OB1:OB1 + 4] = chunkcols(be1)
        bias_all[:, OB2:OB2 + 2] = chunkcols(be2)
        wd1k = np.asarray(Wd1, np.float32)[k]
        bias_all[:, OD1:OD1 + 2] = chunkcols(
            wd1k.T @ be3f + np.asarray(bd1, np.float32)[k])
        bias_all[:, OD2:OD2 + 2] = chunkcols(np.asarray(bd2, np.float32)[k])
        bias_all[:, OD3:OD3 + 1] = chunkcols(np.asarray(bd3, np.float32)[k])
        m = dict(enc)
        m["fT"] = np.ascontiguousarray(f.T).astype(ml_dtypes.bfloat16)
        m["wd1"] = We3f @ wd1k  # encoder L3 folded into decoder layer 1
        m["wd2"] = np.asarray(Wd2, np.float32)[k]
        m["wd3"] = np.asarray(Wd3, np.float32)[k]
        m["bias_all"] = bias_all
        maps.append(m)
    return maps, rows_per_core


_NC_CACHE = {}


def _get_nc(cap=4352):
    if cap not in _NC_CACHE:
        _NC_CACHE[cap] = _build_nc(cap)
    return _NC_CACHE[cap]


def kernel(**inputs) -> np.ndarray:
    cat = np.asarray(inputs["cat_idx"]).astype(np.int64)
    counts = np.bincount(cat, minlength=N_CORES)
    cap = max(256, int(-(-counts.max() // 256) * 256))
    maps, rows_per_core = _pack_inputs(**inputs, cap=cap)
    nc = _get_nc(cap)
    res = bass_utils.run_bass_kernel_spmd(nc, maps, core_ids=list(range(N_CORES)))
    latent = np.zeros((B, LAT), np.float32)
    for k, r in enumerate(res.results):
        rows = rows_per_core[k]
        latent[rows] = r["out"][:, :len(rows)].T
    return latent


# revision 32
# speedup vs baseline: 1.1056x; 1.0336x over previous
"""Trainium2 Bass kernel for nn_CategorySpecificInitNet (moe_routing).

kernel(**inputs) takes the FULL unsharded inputs (keys as in
reference.setup_inputs()) and returns the FULL [B, 128] float32 output.

Strategy — expert-parallel, per the spec sharding hint's dispatch-by-category
alternative:
  - the host sharding layer dispatches rows to cores by category: rows are
    stably sorted by cat_idx and core k receives category k's rows,
    zero-padded to a static per-core capacity (max category count rounded
    up to 256);
  - every core runs the shared encoder plus exactly ONE decoder (its
    category's), so the decoder FLOPs drop 8x vs computing all decoders
    densely;
  - the encoder's linear third layer is constant-folded into the decoder's
    first layer on the host (W_f = We3 @ Wd1_k, b_f = Wd1_k^T be3 + bd1_k —
    exact algebra, ~0.1% of the FLOPs);
  - all compute is feature-major [features(partitions), rows(free)], so no
    transposes are needed on device (the host passes features
    pre-transposed); outputs come back [128, cap] and the host
    inverse-permutes rows during unsharding.

Schedule (validated against the TimelineSim cost model; 94.3us -> 85.3us):
  - the whole pipeline runs in bf16 (f32 PSUM accumulation, f32 biases;
    5.7e-3 max rel error vs the 2e-2 gate): bf16 matmuls run at the same
    1 cycle/row as fp32r but with no >=256 free-size restriction, halve all
    DMA traffic, and let the capacity round to 128 rows (4224 vs 4352);
  - the PE clock ramps to full speed only after ~3us of continuous busy; a
    chain of tiny warmup matmuls on a memset tile anchors the ramp at
    t~0.3us so every real matmul runs at 2.4GHz;
  - all DMA transfers serialize on one pipe, so issue order is programmed
    around need-by times: We1 chunk 0 rides Pool's SWDGE (fastest to first
    byte), the first feature tile streams in (1,2,3) chunk groups on SP,
    and the remaining weights queue strictly behind them; the first real
    matmul issues at ~3.7us instead of ~7.3us;
  - L1 accumulates c-outer/m-inner in 4 concurrent PSUM banks so compute
    starts as soon as chunk 0 lands;
  - per-step PE order is [L1 t][d2+d3 t-1][L2 t][d1 t]: the previous tile's
    decoder matmuls hide the L1 activation latency;
  - the 128-row remainder tile runs LAST so only its ~0.3us decoder chain
    and a tiny store (on the then-idle SP HWDGE queue) trail the last full
    tile's matmuls.
"""
import sys

for _p in ("/opt/trn_rl_repo",):
    if _p not in sys.path:
        sys.path.append(_p)

import numpy as np
import ml_dtypes

import concourse.bass as bass
import concourse.bacc as bacc
import concourse.mybir as mybir
import concourse.tile as tile
from concourse import bass_utils

FR = mybir.dt.float32r
F32 = mybir.dt.float32
BF16 = mybir.dt.bfloat16
Alu = mybir.AluOpType
ActF = mybir.ActivationFunctionType

B, C, H1, H2, HO = 32768, 768, 512, 256, 256
DH, LAT, K = 256, 128, 8
N_CORES = 8
TILE = 512
# bias_all columns: be1[4] be2[2] bf[2](=Wd1^T be3 + bd1) bd2[2] bd3[1]
OB1, OB2, OD1, OD2, OD3 = 0, 4, 6, 8, 10
NBIAS = 11


def _build_nc(cap, tile_n=512, n_warm=10, ps_w_bufs=7, dp_bufs=3, ps_o_bufs=1,
              ap_bufs=4, fp_bufs=3):
    assert cap % 128 == 0
    tiles = [tile_n] * (cap // tile_n)
    if cap % tile_n:
        tiles.append(cap % tile_n)  # remainder tile LAST (shortest tail)
    offs = [sum(tiles[:i]) for i in range(len(tiles))]
    nt = len(tiles)
    nc = bacc.Bacc(name="catnet_ep")

    fT = nc.dram_tensor("fT", (C, cap), BF16, kind="ExternalInput")
    we1 = nc.dram_tensor("we1", (C, H1), BF16, kind="ExternalInput")
    we2 = nc.dram_tensor("we2", (H1, H2), BF16, kind="ExternalInput")
    wd1 = nc.dram_tensor("wd1", (H2, DH), BF16, kind="ExternalInput")  # We3 @ Wd1
    wd2 = nc.dram_tensor("wd2", (DH, DH), BF16, kind="ExternalInput")
    wd3 = nc.dram_tensor("wd3", (DH, LAT), BF16, kind="ExternalInput")
    bias_all = nc.dram_tensor("bias_all", (128, NBIAS), F32, kind="ExternalInput")
    out = nc.dram_tensor("out", (LAT, cap), BF16, kind="ExternalOutput")

    nC, nH1, nH2, nDH = C // 128, H1 // 128, H2 // 128, DH // 128

    with tile.TileContext(nc) as tc:
        with (
            tc.tile_pool(name="wp", bufs=1) as wp,
            tc.tile_pool(name="fp", bufs=fp_bufs) as fp,
            tc.tile_pool(name="ap", bufs=ap_bufs) as ap,
            tc.tile_pool(name="dp", bufs=dp_bufs) as dp,
            tc.tile_pool(name="ps_w", bufs=ps_w_bufs, space="PSUM") as ps_w,
            tc.tile_pool(name="ps_o", bufs=ps_o_bufs, space="PSUM") as ps_o,
        ):
            # --- PE warmup: anchor the p-state ramp at ~t=0.3us so real
            # matmuls (arriving ~3.5us) run at full clock ---
            warm_sb = wp.tile([128, 256], F32, tag="warm_sb")
            nc.vector.memset(warm_sb[:], 0.0)
            warm_fr = warm_sb.bitcast(FR)
            for _ in range(n_warm):
                warm_ps = ps_o.tile([128, 256], F32, tag="out")
                nc.tensor.matmul(warm_ps[:], warm_fr[:, :128], warm_fr[:],
                                 start=True, stop=True)

            # --- weight + bias DMAs ---
            # queue order controls the (serialized) DMA pipe: ACT carries We1
            # chunks then the small tail weights; SP carries the t0 feature
            # chunks then We2/Wd1 (needed next) then the feature tiles.
            # Tensors are declared here; dma_starts are issued interleaved
            # with emit_l1(0) below so nothing preempts the startup chunks.
            we1_t = wp.tile([128, nC, H1], BF16, tag="we1")
            we1_r = we1.rearrange("(c p) h -> p c h", p=128)
            # chunk 0 via Pool SWDGE: ~60ns SEQ issue vs ACT's 667, so the
            # first weight chunk lands ~0.6us earlier
            nc.gpsimd.dma_start(we1_t[:, 0:1, :], we1_r[:, 0:1, :])
            nc.scalar.dma_start(we1_t[:, 1:3, :], we1_r[:, 1:3, :])
            nc.scalar.dma_start(we1_t[:, 3:6, :], we1_r[:, 3:6, :])
            bias_t = wp.tile([128, NBIAS], F32, tag="bias")
            nc.scalar.dma_start(bias_t[:], bias_all[:])
            we2_t = wp.tile([128, nH1, H2], BF16, tag="we2")
            wd1_t = wp.tile([128, nH2, DH], BF16, tag="wd1")
            wd2_t = wp.tile([128, nDH, DH], BF16, tag="wd2")
            wd3_t = wp.tile([128, nDH, LAT], BF16, tag="wd3")

            def emit_weight_dmas():
                nc.sync.dma_start(we2_t[:],
                                  we2.rearrange("(c p) h -> p c h", p=128))
                nc.sync.dma_start(wd1_t[:],
                                  wd1.rearrange("(c p) d -> p c d", p=128))

            def emit_weight_dmas2():
                nc.sync.dma_start(wd2_t[:],
                                  wd2.rearrange("(c p) d -> p c d", p=128))
                nc.sync.dma_start(wd3_t[:],
                                  wd3.rearrange("(c p) d -> p c d", p=128))

            nact = [0]  # alternate pointwise ops between ACT and DVE

            def pact(x, pw, bias_col, relu, split=False, eng=None):
                if eng is not None:
                    if relu:
                        if eng == "act":
                            nc.scalar.activation(x[:], pw[:], ActF.Relu,
                                                 bias=bias_col)
                        else:
                            nc.vector.tensor_scalar(x[:], pw[:], bias_col,
                                                    0.0, Alu.add, Alu.max)
                    else:
                        if eng == "act":
                            nc.scalar.activation(x[:], pw[:], ActF.Identity,
                                                 bias=bias_col)
                        else:
                            nc.vector.tensor_scalar(x[:], pw[:], bias_col,
                                                    0.0, Alu.add, Alu.bypass)
                    return
                if split:
                    # halve latency: ACT and DVE process half-columns each
                    h = x.shape[-1] // 2
                    if relu:
                        nc.scalar.activation(x[:, :h], pw[:, :h], ActF.Relu,
                                             bias=bias_col)
                        nc.vector.tensor_scalar(x[:, h:], pw[:, h:], bias_col,
                                                0.0, Alu.add, Alu.max)
                    else:
                        nc.scalar.activation(x[:, :h], pw[:, :h], ActF.Identity,
                                             bias=bias_col)
                        nc.vector.tensor_scalar(x[:, h:], pw[:, h:], bias_col,
                                                0.0, Alu.add, Alu.bypass)
                    return
                n = nact[0]
                nact[0] += 1
                if relu:
                    if n % 2 == 0:
                        nc.scalar.activation(x[:], pw[:], ActF.Relu,
                                             bias=bias_col)
                    else:
                        nc.vector.tensor_scalar(x[:], pw[:], bias_col,
                                                0.0, Alu.add, Alu.max)
                else:
                    nc.scalar.activation(x[:], pw[:], ActF.Identity,
                                         bias=bias_col)

            def emit_l1(t):
                tn = tiles[t]
                sl = slice(offs[t], offs[t] + tn)
                ftb_fl = fp.tile([128, nC, tile_n], BF16, tag="ft")
                ftb = ftb_fl[:, :, :tn]
                fr = fT.rearrange("(c p) b -> p c b", p=128)[:, :, sl]
                if t == 0:
                    # chunked DMAs: L1 starts after chunk 0 (~0.4us xfer);
                    # (1,2,3) split balances per-DMA issue overhead (~0.7us)
                    # against time-to-first-matmul
                    c0 = 0
                    for w in (1, 2, 3):
                        nc.sync.dma_start(ftb[:, c0:c0 + w, :], fr[:, c0:c0 + w, :])
                        c0 += w
                    emit_weight_dmas()  # behind the t0 chunks on the pipe
                    if nt == 1:
                        emit_weight_dmas2()
                elif t == 1:
                    nc.sync.dma_start(ftb[:, :3, :], fr[:, :3, :])
                    nc.sync.dma_start(ftb[:, 3:, :], fr[:, 3:, :])
                    emit_weight_dmas2()  # wd2/wd3 queue behind t1's chunks
                else:
                    nc.sync.dma_start(ftb[:], fr)
                # c-outer / m-inner: 4 concurrent PSUM accumulation groups,
                # compute starts as soon as c-chunk 0 lands
                pws = []
                for m in range(nH1):
                    pw_fl = ps_w.tile([128, tile_n], F32, tag="pw", name=f"pw1_{m}")
                    pws.append(pw_fl[:, :tn])
                for c in range(nC):
                    for m in range(nH1):
                        nc.tensor.matmul(pws[m][:], we1_t[:, c, bass.ts(m, 128)],
                                         ftb[:, c, :],
                                         start=(c == 0), stop=(c == nC - 1))
                a1 = []
                for m in range(nH1):
                    x = ap.tile([128, tile_n], BF16, tag=f"a1_{m}", name="a1x")[:, :tn]
                    pact(x, pws[m], bias_t[:, OB1 + m:OB1 + m + 1], True)
                    a1.append(x)
                return a1

            def emit_l2(t, a1, split=False, eng=None):
                tn = tiles[t]
                a2 = []
                for m in range(nH2):
                    pw = ps_w.tile([128, tile_n], F32, tag="pw", name="pw")[:, :tn]
                    for c in range(nH1):
                        nc.tensor.matmul(pw[:], we2_t[:, c, bass.ts(m, 128)],
                                         a1[c][:],
                                         start=(c == 0), stop=(c == nH1 - 1))
                    x = ap.tile([128, tile_n], BF16, tag=f"a2_{m}", name="a2x")[:, :tn]
                    pact(x, pw, bias_t[:, OB2 + m:OB2 + m + 1], True, split, eng)
                    a2.append(x)
                return a2

            def emit_d1(t, h, split=False, hold=False, eng=None):
                tn = tiles[t]
                tg = "d1h_" if hold else "d1_"
                bf = 1 if hold else None
                d1 = []
                for m in range(nDH):
                    pw = ps_w.tile([128, tile_n], F32, tag="pw", name="pw")[:, :tn]
                    for c in range(nH2):
                        nc.tensor.matmul(pw[:], wd1_t[:, c, bass.ts(m, 128)],
                                         h[c][:],
                                         start=(c == 0), stop=(c == nH2 - 1))
                    x = dp.tile([128, tile_n], BF16, tag=f"{tg}{m}", name="d1x",
                                bufs=bf)[:, :tn]
                    pact(x, pw, bias_t[:, OD1 + m:OD1 + m + 1], True, split, eng)
                    d1.append(x)
                return d1

            def emit_d2(t, d1, split=False, eng=None):
                tn = tiles[t]
                d2 = []
                for m in range(nDH):
                    pw = ps_w.tile([128, tile_n], F32, tag="pw", name="pw")[:, :tn]
                    for c in range(nDH):
                        nc.tensor.matmul(pw[:], wd2_t[:, c, bass.ts(m, 128)],
                                         d1[c][:],
                                         start=(c == 0), stop=(c == nDH - 1))
                    x = dp.tile([128, tile_n], BF16, tag=f"d2_{m}", name="d2x")[:, :tn]
                    pact(x, pw, bias_t[:, OD2 + m:OD2 + m + 1], True, split, eng)
                    d2.append(x)
                return d2

            def emit_d3_store(t, d2, split=False, eng=None):
                tn = tiles[t]
                last = t == nt - 1
                po = ps_o.tile([128, tile_n], F32, tag="out", name="po")[:, :tn]
                for c in range(nDH):
                    nc.tensor.matmul(po[:], wd3_t[:, c, :], d2[c][:],
                                     start=(c == 0), stop=(c == nDH - 1))
                osb = ap.tile([128, tile_n], BF16, tag="osb", name="osb")[:, :tn]
                pact(osb, po, bias_t[:, OD3:OD3 + 1], False, split, eng)
                # the last store rides the idle SP HWDGE queue (fast issue)
                eng = nc.sync if last else nc.gpsimd
                eng.dma_start(out[:, offs[t]:offs[t] + tn], osb[:])

            def emit_d2_d3_store(t, d1, eng=None, split=False):
                emit_d3_store(t, emit_d2(t, d1, split, eng=eng), split, eng=eng)

            # decoder tail of tile t-2 runs between L1(t) and L2(t): those
            # matmuls are dependency-free and hide the L1 activation latency;
            # depth 2 keeps ready PE work in the endgame so the final tile's
            # act latencies stay covered
            pend = []
            for t in range(nt - 1):
                a1 = emit_l1(t)
                if len(pend) >= 1:
                    emit_d2_d3_store(*pend.pop(0))
                a2 = emit_l2(t, a1)
                pend.append((t, emit_d1(t, a2)))
            # endgame: L1(T); d2d3(T-2); L2(T); d2d3(T-1); d1(T); d2d3(T) —
            # the pending full tiles' decoders cover the tail tile's act
            # latencies, and only the short tail store trails the last matmul
            # endgame engine split: the previous tile's (long) decoder acts
            # go to DVE, the tail tile's (short) act chain to ACT, so the
            # tail's stage latencies never queue behind 0.6us full-width acts
            tl_ = nt - 1
            a1 = emit_l1(tl_)
            if pend:
                emit_d2_d3_store(*pend.pop(0))
            a2 = emit_l2(tl_, a1)
            if pend:
                emit_d2_d3_store(*pend.pop(0))
            d1 = emit_d1(tl_, a2)
            emit_d2_d3_store(tl_, d1)

    nc.finalize()
    return nc


def _pack_inputs(features, We1, be1, We2, be2, We3, be3,
                 Wd1, bd1, Wd2, bd2, Wd3, bd3, cat_idx, cap):
    """Dispatch rows to cores by category (expert-parallel sharding)."""
    features = np.asarray(features, np.float32)
    cat = np.asarray(cat_idx).astype(np.int64)
    order = np.argsort(cat, kind="stable")
    counts = np.bincount(cat, minlength=N_CORES)
    starts = np.zeros(N_CORES + 1, np.int64)
    np.cumsum(counts, out=starts[1:])

    def chunkcols(b):
        b = np.asarray(b, np.float32).reshape(-1)
        return b.reshape(-1, 128).T

    enc = dict(
        we1=np.asarray(We1, np.float32).astype(ml_dtypes.bfloat16),
        we2=np.asarray(We2, np.float32).astype(ml_dtypes.bfloat16),
    )
    We3f = np.asarray(We3, np.float32)
    be3f = np.asarray(be3, np.float32)
    maps, rows_per_core = [], []
    for k in range(N_CORES):
        rows = order[starts[k]:starts[k + 1]]
        rows_per_core.append(rows)
        f = np.zeros((cap, C), np.float32)
        f[:len(rows)] = features[rows]
        bias_all = np.zeros((128, NBIAS), np.float32)
        bias_all[:, OB1:OB1 + 4] = chunkcols(be1)
        bias_all[:, OB2:OB2 + 2] = chunkcols(be2)
        wd1k = np.asarray(Wd1, np.float32)[k]
        bias_all[:, OD1:OD1 + 2] = chunkcols(
            wd1k.T @ be3f + np.asarray(bd1, np.float32)[k])
        bias_all[:, OD2:OD2 + 2] = chunkcols(np.asarray(bd2, np.float32)[k])
        bias_all[:, OD3:OD3 + 1] = chunkcols(np.asarray(bd3, np.float32)[k])
        m = dict(enc)
        m["fT"] = np.ascontiguousarray(f.T).astype(ml_dtypes.bfloat16)
        m["wd1"] = (We3f @ wd1k).astype(ml_dtypes.bfloat16)  # folded L3
        m["wd2"] = np.asarray(Wd2, np.float32)[k].astype(ml_dtypes.bfloat16)
        m["wd3"] = np.asarray(Wd3, np.float32)[k].astype(ml_dtypes.bfloat16)
        m["bias_all"] = bias_all
        maps.append(m)
    return maps, rows_per_core


_NC_CACHE = {}


def _get_nc(cap=4224):
    if cap not in _NC_CACHE:
        _NC_CACHE[cap] = _build_nc(cap)
    return _NC_CACHE[cap]


def kernel(**inputs) -> np.ndarray:
    cat = np.asarray(inputs["cat_idx"]).astype(np.int64)
    counts = np.bincount(cat, minlength=N_CORES)
    cap = max(256, int(-(-counts.max() // 128) * 128))
    maps, rows_per_core = _pack_inputs(**inputs, cap=cap)
    nc = _get_nc(cap)
    res = bass_utils.run_bass_kernel_spmd(nc, maps, core_ids=list(range(N_CORES)))
    latent = np.zeros((B, LAT), np.float32)
    for k, r in enumerate(res.results):
        rows = rows_per_core[k]
        latent[rows] = r["out"][:, :len(rows)].T.astype(np.float32)
    return latent

